# revision 38
# baseline (speedup 1.0000x reference)
"""GATv2Stack Trainium2 kernel (8-core data-parallel over graphs), v3.

bt=128 graphs of N=64 nodes, 16 graphs/core. See reference.py.
  h = x @ W_in + b_in
  2x: xl=h@Wl+bl; xr=h@Wr+br; e=att.lrelu(xr_i+xl_j); a=softmax_j(e+mask)
      g = a@(h@Wl) + (out_bias+bl); g=ELU(g); g=LN(g); h=g+h
  out = where(keep_graph, h, x@W_in+b_in)

v3 design (from v2 trace: Scalar 58%, Vector 57%, DMA queue time ~165us):
  - w-factorization: exp(e'-4) = E_ij * w_j with
      E = exp(0.8*att.max(-xl_j, xr_i) - 2)   [fused into Act psum evac]
      w_j = exp((att.xl)_j + mask_j - 2)      [tiny Act exp of flipped pax]
    attention-out moving operand = w*xlOb (+w cols for Z), so the DVE
    scatter-add (STT) and separate exp pass are deleted entirely.
  - e-scatter DMAs grouped over equal-m gp runs: one DMA per
    (group, head, par, t) instead of per (gp, ...): ~64 -> ~24-32/layer;
    ALL DMAs issued on sync queue only (scalar SEQ freed for Act work).
  - all XBAR DMA transposes (xn0, h_node, hT) -> PE transposes + evacs
  - pairwise-MAX (dominant DVE op) split DVE/GpSimd ~4:3
  - rz (1/Z) folded into gn psum evac as per-partition Act scale
  - ELU's -1 dropped (LN-invariant); sum(x^2) via Act accum_out
Per-core layouts (G=16 graphs, gp pair idx, par=g%2):
  hT[m]     [128,1024] f16  [m*128+c, g*64+node]
  h_node    [128,2048] f16  [par*64+node, gp*256+ch]
  xlTn/xrTb/xlOb[hp] [128,1024] f16 (t,c) x (g,node)
  sl (gp,hp) [128,2*m*m]  f16 cols par*m*m + j*m + i
  e_all      [128, sum(mm)] f16 rows {32s+t}, cols eoff[gp]+par*... E vals
  aE_w      [128,2048] f16  [par*64+j, gp*256 + h*64?? no: gp*512/2..]
            actually [par*64+j, gp*512 + h*128 + par*64 + i] f16 = E
  xn0       [128, 8*320+8] f16 [par*64+node, gp*320 + hp*128 + t*64 + c],
            cols gp*320+256..260 = w_j per head
"""
import sys
sys.path.insert(0, '/opt/trn_rl_repo')
import numpy as np

import concourse.bass as bass
import concourse.mybir as mybir
from concourse import bass_utils, bacc
from concourse.tile import TileContext

dt = mybir.dt
F32, F16 = dt.float32, dt.float16
AF = mybir.ActivationFunctionType
ALU = mybir.AluOpType

B, T, N, D_IN = 2, 64, 64, 512
HID, L, H, C = 256, 2, 4, 64
BT = B * T
G = 16
NCORES = 8
LN_EPS = 1e-5
NEG_BIG = -30000.0
WB = 2.0  # bias split: E=exp(0.8*attmax-2), w=exp(attxl+mask-2)

_n = [0]
def _nm(p="t"):
    _n[0] += 1
    return f"{p}{_n[0]}"


def fd(ap, *dims):
    """Keep partition dim + offset of (sliced) AP, replace free dims."""
    return bass.AP(ap.tensor, ap.offset, [list(ap.ap[0])] + [[s, c] for (s, c) in dims])


def _chunking(m):
    """Uniform i-chunks: smallest even nch with (m/nch)*m <= 512."""
    nch = 2
    while (m // nch) * m > 512 or m % nch != 0:
        nch += 2
    return nch, m // nch


def build_nc(mh=(64,) * G):
    nc = bacc.Bacc("TRN2", target_bir_lowering=False, debug=False,
                   enable_asserts=False, num_devices=1)

    def din(name, shape, dtp=F16):
        return nc.dram_tensor(name, list(shape), dtp, kind="ExternalInput").ap()

    # merged inputs: few big DMAs instead of ~34 small ones
    xT_d    = din("xTm", [128, 4 * G * 64])          # 4 d-chunks side by side
    win_d   = din("winm", [128, 4 * HID])            # 4 d-chunks
    wlr_d   = din("wlrm", [128, 8 * HID])            # wl(l,k) 4x256, wr 4x256
    cf32_d  = din("cf32", [128, 150], F32)  # binT2 nblT4 oblT4 brT4 att128 attN8
    cf16_d  = din("cf16", [128, 2 * HID + 2 * HID + 128])  # gam, bet, idn
    mbT_d   = din("mbT", [128, 8], F32)              # mask - WB, [par*64+j, gp]
    out_d   = nc.dram_tensor("out", [G * 64, HID], F16, kind="ExternalOutput").ap()

    # per-gp m and equal-m groups (consecutive)
    gpm = [mh[2 * gp] for gp in range(8)]
    groups = []
    s0 = 0
    for gp in range(1, 9):
        if gp == 8 or gpm[gp] != gpm[s0]:
            groups.append((s0, gp))
            s0 = gp
    # e_all col layout: per gp block of mm cols = j*m+i; par lives in the
    # psum/e_all ROW (32*(2hp+par)+t), not in a column offset.
    eoff = [0] * 9
    for gp in range(8):
        eoff[gp + 1] = eoff[gp] + gpm[gp] * gpm[gp]
    etot = eoff[8]

    # engine rotation for small psum->sbuf evacs
    evc = [0]
    def evace():
        # psum -> sbuf evacs: GpSimd cannot access PSUM on TRN2
        evc[0] += 1
        return lambda dst, src: nc.scalar.activation(dst, src, AF.Identity)
    def maxe():
        return nc.vector

    with TileContext(nc) as tc:
        with tc.tile_pool(name="const", bufs=1) as cpool, \
             tc.tile_pool(name="wide", bufs=1) as wpool, \
             tc.tile_pool(name="slp", bufs=1) as slpool, \
             tc.tile_pool(name="sm", bufs=2) as smpool, \
             tc.tile_pool(name="psum", bufs=1, space="PSUM") as ppool:

            def ctile(name, dram_ap, shape, dtp=F16, eng=None):
                t0 = cpool.tile(shape, dtp, name=_nm(name))
                (eng or nc.gpsimd).dma_start(t0[:], dram_ap)
                return t0

            winm = ctile("winm", win_d, [128, 4 * HID], eng=nc.sync)
            win = [winm[:, k * HID:(k + 1) * HID] for k in range(4)]
            cf32 = ctile("cf32", cf32_d, [128, 150], F32, eng=nc.scalar)
            wlrm = ctile("wlrm", wlr_d, [128, 8 * HID], eng=nc.scalar)
            wl = [[wlrm[:, (l * 2 + k) * HID:(l * 2 + k + 1) * HID]
                   for k in range(2)] for l in range(L)]
            wr = [[wlrm[:, 4 * HID + (l * 2 + k) * HID:
                        4 * HID + (l * 2 + k + 1) * HID]
                   for k in range(2)] for l in range(L)]
            binT = cf32[:, 0:2]
            nblT = cf32[:, 2:6]
            oblT = cf32[:, 6:10]
            brT = cf32[:, 10:14]
            att10 = cpool.tile([128, 32 * 2 * L], F16, name=_nm("att10"))
            nc.vector.tensor_copy(att10[:], cf32[:, 14:142])
            attN = cpool.tile([128, 4 * L], F16, name=_nm("attN"))
            nc.vector.tensor_copy(attN[:], cf32[:, 142:150])
            cf16 = ctile("cf16", cf16_d, [128, 4 * HID + 128], eng=nc.scalar)
            gam = [cf16[:, l * HID:(l + 1) * HID] for l in range(L)]
            bet = [cf16[:, 2 * HID + l * HID:2 * HID + (l + 1) * HID]
                   for l in range(L)]
            idn = cf16[:, 4 * HID:4 * HID + 128]
            mbT = ctile("mbT", mbT_d, [128, 8], F32)
            epsb = cpool.tile([128, 1], F32, name=_nm("epsb"))
            nc.vector.memset(epsb[:], LN_EPS)
            ebias = cpool.tile([128, 1], F32, name=_nm("ebias"))
            nc.vector.memset(ebias[:], -WB)

            # aE: exp'd logits; cross-par / pad cells must be EXACTLY 0
            # (they sit inside attention-out stationary slabs).
            aE_w = wpool.tile([128, 16 * HID], F16, name=_nm("aew"), tag="aew")
            nc.gpsimd.memset(aE_w[:, 0:2048], 0.0)
            nc.vector.memset(aE_w[:, 2048:4096], 0.0)

            # ---------- input: load xT (d-major), project ----------
            hT = [smpool.tile([128, G * 64], F16, name=_nm("hT"), tag=f"hT{m}", bufs=1)
                  for m in range(2)]
            with tc.tile_pool(name="xtp", bufs=1) as xtpool:
                xTm = xtpool.tile([128, 4 * G * 64], F16, name=_nm("xT"))
                for hh in range(2):
                    nc.sync.dma_start(xTm[:, hh * 2048:(hh + 1) * 2048],
                                      xT_d[:, hh * 2048:(hh + 1) * 2048])
                xT = [xTm[:, k * 1024:(k + 1) * 1024] for k in range(4)]
                for cb in range(2):
                    for m in range(2):
                        ph = ppool.tile([128, 512], F32, name=_nm("ph"), tag="pps", bufs=2)
                        for k in range(4):
                            nc.tensor.matmul(ph[:], win[k][:, m * 128:(m + 1) * 128],
                                             xT[k][:, cb * 512:(cb + 1) * 512],
                                             start=(k == 0), stop=(k == 3))
                        nc.scalar.activation(hT[m][:, cb * 512:(cb + 1) * 512], ph[:],
                                             AF.Identity, bias=binT[:, m:m + 1])

            # h_node via PE transposes
            h_node_w = smpool.tile([128, 8 * HID], F16, name=_nm("hnode"), tag="hnode",
                                   bufs=2)
            for gp in range(8):
                for m in range(2):
                    tp = ppool.tile([128, 128], F16, name=_nm("tp"), tag="tp", bufs=2)
                    nc.tensor.transpose(tp[:], hT[m][:, gp * 128:(gp + 1) * 128],
                                        idn[:])
                    evace()(
                        h_node_w[:, gp * HID + m * 128:gp * HID + m * 128 + 128],
                        tp[:])

            # ---------- layers ----------
            for l in range(L):
                xrTb = [smpool.tile([128, G * 64], F16, name=_nm("xrTb"), tag=f"xrTb{m}",
                                    bufs=1) for m in range(2)]
                xlTn = [smpool.tile([128, G * 64], F16, name=_nm("xlTn"), tag=f"xlTn{m}",
                                    bufs=1) for m in range(2)]
                xlOb = [smpool.tile([128, G * 64], F16, name=_nm("xlOb"), tag=f"xlOb{m}",
                                    bufs=1) for m in range(2)]
                # cb-major: all of chunk cb=0 (gps 0-3) finishes first so the
                # first gps' sl-MAX starts as early as possible
                for cb in range(2):
                    sl_ = (slice(None), slice(cb * 512, (cb + 1) * 512))
                    for m in range(2):
                        pp = ppool.tile([128, 512], F32, name=_nm("pp"), tag="pps", bufs=2)
                        for k in range(2):
                            nc.tensor.matmul(pp[:], wl[l][k][:, m * 128:(m + 1) * 128],
                                             hT[k][:, cb * 512:(cb + 1) * 512],
                                             start=(k == 0), stop=(k == 1))
                        nc.scalar.activation(xlTn[m][sl_], pp[:], AF.Identity,
                                             bias=nblT[:, l * 2 + m:l * 2 + m + 1],
                                             scale=-1.0)
                    for m in range(2):
                        pp = ppool.tile([128, 512], F32, name=_nm("pp"), tag="pps", bufs=2)
                        for k in range(2):
                            nc.tensor.matmul(pp[:], wr[l][k][:, m * 128:(m + 1) * 128],
                                             hT[k][:, cb * 512:(cb + 1) * 512],
                                             start=(k == 0), stop=(k == 1))
                        nc.scalar.activation(
                            xrTb[m][:, cb * 512:(cb + 1) * 512], pp[:], AF.Identity,
                            bias=brT[:, l * 2 + m:l * 2 + m + 1])
                    for m in range(2):
                        pp = ppool.tile([128, 512], F32, name=_nm("pp"), tag="pps", bufs=2)
                        for k in range(2):
                            nc.tensor.matmul(pp[:], wl[l][k][:, m * 128:(m + 1) * 128],
                                             hT[k][:, cb * 512:(cb + 1) * 512],
                                             start=(k == 0), stop=(k == 1))
                        nc.scalar.activation(xlOb[m][sl_], pp[:], AF.Identity,
                                             bias=oblT[:, l * 2 + m:l * 2 + m + 1])

                # ---- w_j = exp(att.xl + mask - WB) via flipped pax ----
                wT = smpool.tile([128, 32], F32, name=_nm("wT"), tag="wT", bufs=2)
                for gp in range(8):
                    paxp = ppool.tile([128, 512], F32, name=_nm("paxp"), tag="ops",
                                      bufs=2)
                    for par in range(2):
                        g = gp * 2 + par
                        for hp in range(2):
                            nc.tensor.matmul(
                                paxp[par * 64:par * 64 + 64, hp * 2:hp * 2 + 2],
                                xlTn[hp][:, g * 64:g * 64 + 64],
                                attN[:, l * 4 + hp * 2:l * 4 + hp * 2 + 2],
                                start=True, stop=True,
                                tile_position=(0, 64 * par))
                    nc.scalar.activation(wT[:, gp * 4:gp * 4 + 4], paxp[:, 0:4],
                                         AF.Exp, bias=mbT[:, gp:gp + 1])

                # ---- xn0 = w * xlOb node-major (PE transpose) + w cols ----
                xn0 = smpool.tile([128, 8 * 320 + 64], F16, name=_nm("xn"), tag="xn0",
                                  bufs=1)
                for gp in range(8):
                    for hp in range(2):
                        tp = ppool.tile([128, 128], F16, name=_nm("tp"), tag="tp",
                                        bufs=2)
                        nc.tensor.transpose(tp[:], xlOb[hp][:, gp * 128:(gp + 1) * 128],
                                            idn[:])
                        evace()(xn0[:, gp * 320 + hp * 128:gp * 320 + hp * 128 + 128],
                                tp[:])
                    evace()(xn0[:, gp * 320 + 256:gp * 320 + 260],
                            wT[:, gp * 4:gp * 4 + 4])
                # scale xlOb rows by w_j: one wide op, per (gp, h) 64-col block
                nc.vector.tensor_tensor(
                    fd(xn0[0:128, 0:1], (320, 8), (64, 4), (1, 64)),
                    fd(xn0[0:128, 0:1], (320, 8), (64, 4), (1, 64)),
                    fd(wT[0:128, 0:1], (4, 8), (1, 4), (0, 64)), op=ALU.mult)

                # ---- attention: E = exp(0.8*att.max - WB) ----
                e_all = wpool.tile([128, etot], F16, name=_nm("eall"), tag="eall")
                for gp in range(8):
                    m = gpm[gp]
                    mm = m * m
                    nch, ipc = _chunking(m)
                    w = ipc * m
                    # sl tiles per (hp): cols par*mm + j*m + i  (j-major)
                    slts = []
                    for hp in range(2):
                        slt = slpool.tile([128, 2 * mm], F16, name=_nm("sl"), tag="sl",
                                          bufs=3, padded_shape=[128, 2 * 64 * 64])
                        for par in range(2):
                            g = gp * 2 + par
                            dst = fd(slt[:, par * mm:par * mm + 1], (m, m), (1, m))
                            xr_sl = xrTb[hp][:, g * 64:g * 64 + 1]
                            xl_sl = xlTn[hp][:, g * 64:g * 64 + 1]
                            maxe().tensor_tensor(dst, fd(xl_sl, (1, m), (0, m)),
                                                 fd(xr_sl, (0, m), (1, m)), op=ALU.max)
                        slts.append(slt)
                    # e matmuls: 4 streams share psum rows 32*s+t; Act evac
                    # fuses exp: E = exp(0.8*pe - WB)
                    for ci in range(nch):
                        pe = ppool.tile([128, 512], F32, name=_nm("pe"),
                                        tag="eps", bufs=2)
                        for hp in range(2):
                            for par in range(2):
                                s = 2 * hp + par
                                nc.tensor.matmul(
                                    pe[32 * s:32 * s + 2, 0:w],
                                    att10[:, (l * 2 + hp) * 32:(l * 2 + hp) * 32 + 2],
                                    slts[hp][:, par * mm + ci * w:
                                             par * mm + (ci + 1) * w],
                                    start=True, stop=True,
                                    tile_position=(0, 32 * s))
                        nc.scalar.activation(
                            e_all[:, eoff[gp] + ci * w:eoff[gp] + (ci + 1) * w],
                            pe[:, 0:w], AF.Exp, bias=ebias[:], scale=0.8)

                # ---- scatter: e_all -> aE_w (per gp; DMA APs cap at 3 dims
                # so the equal-m group merge is not expressible SBUF->SBUF) --
                for gp in range(8):
                    m = gpm[gp]
                    mm = m * m
                    for hp in range(2):
                        for par in range(2):
                            s = 2 * hp + par
                            for t in range(2):
                                src = fd(e_all[32 * s + t:32 * s + t + 1,
                                               eoff[gp]:eoff[gp] + 1],
                                         (m, m), (1, m))
                                cb0 = gp * 512 + (2 * hp + t) * 128 + par * 64
                                db = aE_w[par * 64:par * 64 + m, cb0:cb0 + 1]
                                dstp = fd(db, (1, m))
                                (nc.sync if (hp + par + t + gp) % 2 else
                                 nc.gpsimd).dma_start(dstp, src)

                # ---- attention out (node-major) + Z via w cols ----
                gn16 = wpool.tile([128, 8 * HID], F16, name=_nm("gn16"), tag="gn16")
                rz_w = smpool.tile([128, 32], F32, name=_nm("rzw"), tag="rzw", bufs=2)
                tmin = wpool.tile([128, 8 * HID], F16, name=_nm("tmin"), tag="tmin")
                sum_w = smpool.tile([128, 8], F32, name=_nm("sumw"), tag="sumw", bufs=2)
                vs_w = smpool.tile([128, 8], F32, name=_nm("vsw"), tag="vsw", bufs=2)
                sqs = smpool.tile([128, HID], F16, name=_nm("sqs"), tag="sqs", bufs=2)
                for gp in range(8):
                    po = ppool.tile([128, 512], F32, name=_nm("po"), tag="ops", bufs=2)
                    for h_g in range(4):
                        mov = fd(xn0[0:128, gp * 320 + h_g * 64:gp * 320 + h_g * 64 + 1],
                                 (256 - 63 * h_g, 2), (1, 64))
                        nc.tensor.matmul(
                            po[:, h_g * 128:h_g * 128 + 128],
                            aE_w[:, (gp * 4 + h_g) * 128:(gp * 4 + h_g) * 128 + 128],
                            mov, start=True, stop=True)
                    # clamp Z away from 0 (pad columns i>=m have Z=0); the
                    # clamped rz multiplies an exactly-0 numerator -> 0.
                    zsb = smpool.tile([128, 4], F32, name=_nm("zsb"), tag="zsb",
                                      bufs=2)
                    nc.vector.tensor_scalar(zsb[:], fd(po[0:128, 64:65], (128, 4)),
                                            1e-30, None, op0=ALU.max)
                    nc.vector.reciprocal(rz_w[:, gp * 4:gp * 4 + 4], zsb[:])
                    # per-gp tail (pipelines with later gps' attention):
                    # rz folded into per-head Act evac scale; ELU min via
                    # Act Relu (min(x,0) = -Relu(-x)) to relieve DVE
                    sl8 = slice(gp * HID, (gp + 1) * HID)
                    for h_g in range(4):
                        nc.scalar.activation(
                            gn16[:, gp * HID + h_g * 64:gp * HID + h_g * 64 + 64],
                            po[:, h_g * 128:h_g * 128 + 64],
                            AF.Identity, scale=rz_w[:, gp * 4 + h_g:gp * 4 + h_g + 1])
                    nc.scalar.activation(tmin[:, sl8], gn16[:, sl8], AF.Relu,
                                         scale=-1.0)
                    nc.scalar.activation(tmin[:, sl8], tmin[:, sl8], AF.Exp,
                                         scale=-1.0)
                    nc.vector.scalar_tensor_tensor(
                        gn16[:, sl8], gn16[:, sl8], 0.0, tmin[:, sl8],
                        op0=ALU.max, op1=ALU.add, accum_out=sum_w[:, gp:gp + 1])
                    nc.scalar.activation(sqs[:], gn16[:, sl8], AF.Square,
                                         accum_out=vs_w[:, gp:gp + 1])

                # ---- LayerNorm stats (whole-layer; one Sqrt site/layer
                # keeps Act table swaps to one pair per layer) ----
                mu_w = smpool.tile([128, 8], F32, name=_nm("muw"), tag="muw", bufs=2)
                musq = smpool.tile([128, 8], F32, name=_nm("musq"), tag="musq", bufs=2)
                var_w = smpool.tile([128, 8], F32, name=_nm("varw"), tag="varw", bufs=2)
                rstd_w = smpool.tile([128, 8], F32, name=_nm("rstdw"), tag="rstdw",
                                     bufs=2)
                nmr = smpool.tile([128, 8], F32, name=_nm("nmr"), tag="nmr", bufs=2)
                nc.vector.tensor_scalar(mu_w[:], sum_w[:], 1.0 / HID, None,
                                        op0=ALU.mult)
                nc.vector.tensor_tensor(musq[:], mu_w[:], mu_w[:], op=ALU.mult)
                nc.vector.scalar_tensor_tensor(var_w[:], vs_w[:], 1.0 / HID,
                                               musq[:], op0=ALU.mult,
                                               op1=ALU.subtract)
                nc.scalar.activation(var_w[:], var_w[:], AF.Sqrt, bias=epsb[:])
                nc.vector.reciprocal(rstd_w[:], var_w[:])
                nc.vector.scalar_tensor_tensor(nmr[:], mu_w[:], -1.0, rstd_w[:],
                                               op0=ALU.mult, op1=ALU.mult)
                hn_w = smpool.tile([128, 8 * HID], F16, name=_nm("hn"), tag="hnode",
                                   bufs=2)
                if l + 1 < L:
                    hT = [smpool.tile([128, G * 64], F16, name=_nm("hT"), tag=f"hT{m}",
                                      bufs=1) for m in range(2)]
                for gp in range(8):
                    sl8 = slice(gp * HID, (gp + 1) * HID)
                    nc.scalar.activation(gn16[:, sl8], gn16[:, sl8],
                                         AF.Identity, bias=nmr[:, gp:gp + 1],
                                         scale=rstd_w[:, gp:gp + 1])
                    # contiguous per-gp gamma/beta (2x-eligible), then residual
                    nc.vector.tensor_tensor(gn16[:, sl8], gn16[:, sl8],
                                            gam[l][:, :], op=ALU.mult)
                    nc.vector.tensor_tensor(gn16[:, sl8], gn16[:, sl8],
                                            bet[l][:, :], op=ALU.add)
                    nc.vector.tensor_tensor(hn_w[:, sl8], gn16[:, sl8],
                                            h_node_w[:, sl8], op=ALU.add)
                    if l + 1 < L:
                        for m in range(2):
                            tp = ppool.tile([128, 128], F16, name=_nm("tp"), tag="tp",
                                            bufs=2)
                            nc.tensor.transpose(
                                tp[:],
                                hn_w[:, gp * HID + m * 128:gp * HID + m * 128 + 128],
                                idn[:])
                            evace()(hT[m][:, gp * 128:(gp + 1) * 128],
                                    tp[:])
                h_node_w = hn_w

            # ---------- output DMA ----------
            for par in range(2):
                src = fd(h_node_w[par * 64:par * 64 + 64, 0:1], (HID, 8), (1, HID))
                dst_sl = out_d[par * 64:par * 64 + 1, :]
                dst = bass.AP(dst_sl.tensor, dst_sl.offset,
                              [[HID, 64], [2 * 64 * HID, 8], [1, HID]])
                nc.sync.dma_start(dst, src)

    nc.finalize()
    return nc


# revision 39
# speedup vs baseline: 1.1018x; 1.1018x over previous
"""GATv2Stack Trainium2 kernel (8-core data-parallel over graphs), v3.

bt=128 graphs of N=64 nodes, 16 graphs/core. See reference.py.
  h = x @ W_in + b_in
  2x: xl=h@Wl+bl; xr=h@Wr+br; e=att.lrelu(xr_i+xl_j); a=softmax_j(e+mask)
      g = a@(h@Wl) + (out_bias+bl); g=ELU(g); g=LN(g); h=g+h
  out = where(keep_graph, h, x@W_in+b_in)

v3 design (from v2 trace: Scalar 58%, Vector 57%, DMA queue time ~165us):
  - w-factorization: exp(e'-4) = E_ij * w_j with
      E = exp(0.8*att.max(-xl_j, xr_i) - 2)   [fused into Act psum evac]
      w_j = exp((att.xl)_j + mask_j - 2)      [tiny Act exp of flipped pax]
    attention-out moving operand = w*xlOb (+w cols for Z), so the DVE
    scatter-add (STT) and separate exp pass are deleted entirely.
  - e-scatter DMAs grouped over equal-m gp runs: one DMA per
    (group, head, par, t) instead of per (gp, ...): ~64 -> ~24-32/layer;
    ALL DMAs issued on sync queue only (scalar SEQ freed for Act work).
  - all XBAR DMA transposes (xn0, h_node, hT) -> PE transposes + evacs
  - pairwise-MAX (dominant DVE op) split DVE/GpSimd ~4:3
  - rz (1/Z) folded into gn psum evac as per-partition Act scale
  - ELU's -1 dropped (LN-invariant); sum(x^2) via Act accum_out
Per-core layouts (G=16 graphs, gp pair idx, par=g%2):
  hT[m]     [128,1024] f16  [m*128+c, g*64+node]
  h_node    [128,2048] f16  [par*64+node, gp*256+ch]
  xlTn/xrTb/xlOb[hp] [128,1024] f16 (t,c) x (g,node)
  sl (gp,hp) [128,2*m*m]  f16 cols par*m*m + j*m + i
  e_all      [128, sum(mm)] f16 rows {32s+t}, cols eoff[gp]+par*... E vals
  aE_w      [128,2048] f16  [par*64+j, gp*256 + h*64?? no: gp*512/2..]
            actually [par*64+j, gp*512 + h*128 + par*64 + i] f16 = E
  xn0       [128, 8*320+8] f16 [par*64+node, gp*320 + hp*128 + t*64 + c],
            cols gp*320+256..260 = w_j per head
"""
import sys
sys.path.insert(0, '/opt/trn_rl_repo')
import numpy as np

import concourse.bass as bass
import concourse.mybir as mybir
from concourse import bass_utils, bacc
from concourse.tile import TileContext

dt = mybir.dt
F32, F16 = dt.float32, dt.float16
AF = mybir.ActivationFunctionType
ALU = mybir.AluOpType

B, T, N, D_IN = 2, 64, 64, 512
HID, L, H, C = 256, 2, 4, 64
BT = B * T
G = 16
NCORES = 8
LN_EPS = 1e-5
NEG_BIG = -30000.0
WB = 2.0  # bias split: E=exp(0.8*attmax-2), w=exp(attxl+mask-2)

_n = [0]
def _nm(p="t"):
    _n[0] += 1
    return f"{p}{_n[0]}"


def fd(ap, *dims):
    """Keep partition dim + offset of (sliced) AP, replace free dims."""
    return bass.AP(ap.tensor, ap.offset, [list(ap.ap[0])] + [[s, c] for (s, c) in dims])


def _chunking(m):
    """Uniform i-chunks: smallest even nch with (m/nch)*m <= 512."""
    nch = 2
    while (m // nch) * m > 512 or m % nch != 0:
        nch += 2
    return nch, m // nch


def build_nc(mh=(64,) * G):
    nc = bacc.Bacc("TRN2", target_bir_lowering=False, debug=False,
                   enable_asserts=False, num_devices=1)

    def din(name, shape, dtp=F16):
        return nc.dram_tensor(name, list(shape), dtp, kind="ExternalInput").ap()

    # merged inputs: few big DMAs instead of ~34 small ones
    xT_d    = din("xTm", [128, 4 * G * 64])          # 4 d-chunks side by side
    win_d   = din("winm", [128, 4 * HID])            # 4 d-chunks
    wlr_d   = din("wlrm", [128, 8 * HID])            # wl(l,k) 4x256, wr 4x256
    cf32_d  = din("cf32", [128, 150], F32)  # binT2 nblT4 oblT4 brT4 att128 attN8
    cf16_d  = din("cf16", [128, 2 * HID + 2 * HID + 128])  # gam, bet, idn
    mbT_d   = din("mbT", [128, 8], F32)              # mask - WB, [par*64+j, gp]
    out_d   = nc.dram_tensor("out", [G * 64, HID], F16, kind="ExternalOutput").ap()

    # per-gp m and equal-m groups (consecutive)
    gpm = [mh[2 * gp] for gp in range(8)]
    groups = []
    s0 = 0
    for gp in range(1, 9):
        if gp == 8 or gpm[gp] != gpm[s0]:
            groups.append((s0, gp))
            s0 = gp
    # e_all col layout: per gp block of mm cols = j*m+i; par lives in the
    # psum/e_all ROW (32*(2hp+par)+t), not in a column offset.
    eoff = [0] * 9
    for gp in range(8):
        eoff[gp + 1] = eoff[gp] + gpm[gp] * gpm[gp]
    etot = eoff[8]

    # engine rotation for small psum->sbuf evacs
    evc = [0]
    def evace():
        # psum -> sbuf evacs: GpSimd cannot access PSUM on TRN2
        evc[0] += 1
        return lambda dst, src: nc.scalar.activation(dst, src, AF.Identity)
    def maxe():
        return nc.vector

    with TileContext(nc) as tc:
        with tc.tile_pool(name="const", bufs=1) as cpool, \
             tc.tile_pool(name="wide", bufs=1) as wpool, \
             tc.tile_pool(name="slp", bufs=1) as slpool, \
             tc.tile_pool(name="sm", bufs=2) as smpool, \
             tc.tile_pool(name="psum", bufs=1, space="PSUM") as ppool:

            def ctile(name, dram_ap, shape, dtp=F16, eng=None):
                t0 = cpool.tile(shape, dtp, name=_nm(name))
                (eng or nc.gpsimd).dma_start(t0[:], dram_ap)
                return t0

            winm = ctile("winm", win_d, [128, 4 * HID], eng=nc.sync)
            win = [winm[:, k * HID:(k + 1) * HID] for k in range(4)]
            wlrm = ctile("wlrm", wlr_d, [128, 8 * HID], eng=nc.scalar)
            wl = [[wlrm[:, (l * 2 + k) * HID:(l * 2 + k + 1) * HID]
                   for k in range(2)] for l in range(L)]
            wr = [[wlrm[:, 4 * HID + (l * 2 + k) * HID:
                        4 * HID + (l * 2 + k + 1) * HID]
                   for k in range(2)] for l in range(L)]
            cf32 = ctile("cf32", cf32_d, [128, 150], F32, eng=nc.scalar)
            binT = cf32[:, 0:2]
            nblT = cf32[:, 2:6]
            oblT = cf32[:, 6:10]
            brT = cf32[:, 10:14]
            att10 = cpool.tile([128, 32 * 2 * L], F16, name=_nm("att10"))
            nc.vector.tensor_copy(att10[:], cf32[:, 14:142])
            attN = cpool.tile([128, 4 * L], F16, name=_nm("attN"))
            nc.vector.tensor_copy(attN[:], cf32[:, 142:150])
            cf16 = ctile("cf16", cf16_d, [128, 4 * HID + 128], eng=nc.scalar)
            gam = [cf16[:, l * HID:(l + 1) * HID] for l in range(L)]
            bet = [cf16[:, 2 * HID + l * HID:2 * HID + (l + 1) * HID]
                   for l in range(L)]
            idn = cf16[:, 4 * HID:4 * HID + 128]
            mbT = ctile("mbT", mbT_d, [128, 8], F32)
            epsb = cpool.tile([128, 1], F32, name=_nm("epsb"))
            nc.vector.memset(epsb[:], LN_EPS)
            ebias = cpool.tile([128, 1], F32, name=_nm("ebias"))
            nc.vector.memset(ebias[:], -WB)

            # aE: exp'd logits; cross-par / pad cells must be EXACTLY 0
            # (they sit inside attention-out stationary slabs).
            aE_w = wpool.tile([128, 16 * HID], F16, name=_nm("aew"), tag="aew")
            nc.gpsimd.memset(aE_w[:, 0:2048], 0.0)
            nc.vector.memset(aE_w[:, 2048:4096], 0.0)

            # ---------- input: load xT (d-major), project ----------
            hT = [smpool.tile([128, G * 64], F16, name=_nm("hT"), tag=f"hT{m}", bufs=1)
                  for m in range(2)]
            with tc.tile_pool(name="xtp", bufs=1) as xtpool:
                xTm = xtpool.tile([128, 4 * G * 64], F16, name=_nm("xT"))
                for hh in range(2):
                    nc.sync.dma_start(xTm[:, hh * 2048:(hh + 1) * 2048],
                                      xT_d[:, hh * 2048:(hh + 1) * 2048])
                xT = [xTm[:, k * 1024:(k + 1) * 1024] for k in range(4)]
                for m in range(2):
                    for cb in range(2):
                        ph = ppool.tile([128, 512], F32, name=_nm("ph"), tag="pps", bufs=2)
                        for k in range(4):
                            nc.tensor.matmul(ph[:], win[k][:, m * 128:(m + 1) * 128],
                                             xT[k][:, cb * 512:(cb + 1) * 512],
                                             start=(k == 0), stop=(k == 3))
                        nc.scalar.activation(hT[m][:, cb * 512:(cb + 1) * 512], ph[:],
                                             AF.Identity, bias=binT[:, m:m + 1])

            # h_node via PE transposes
            h_node_w = smpool.tile([128, 8 * HID], F16, name=_nm("hnode"), tag="hnode",
                                   bufs=2)
            for gp in range(8):
                for m in range(2):
                    tp = ppool.tile([128, 128], F16, name=_nm("tp"), tag="tp", bufs=2)
                    nc.tensor.transpose(tp[:], hT[m][:, gp * 128:(gp + 1) * 128],
                                        idn[:])
                    evace()(
                        h_node_w[:, gp * HID + m * 128:gp * HID + m * 128 + 128],
                        tp[:])

            # ---------- layers ----------
            for l in range(L):
                xrTb = [smpool.tile([128, G * 64], F16, name=_nm("xrTb"), tag=f"xrTb{m}",
                                    bufs=1) for m in range(2)]
                xlTn = [smpool.tile([128, G * 64], F16, name=_nm("xlTn"), tag=f"xlTn{m}",
                                    bufs=1) for m in range(2)]
                xlOb = [smpool.tile([128, G * 64], F16, name=_nm("xlOb"), tag=f"xlOb{m}",
                                    bufs=1) for m in range(2)]
                # cb-major: all of chunk cb=0 (gps 0-3) finishes first so the
                # first gps' sl-MAX starts as early as possible
                for cb in range(2):
                    for m in range(2):
                        pp = ppool.tile([128, 512], F32, name=_nm("pp"), tag="pps", bufs=2)
                        for k in range(2):
                            nc.tensor.matmul(pp[:], wl[l][k][:, m * 128:(m + 1) * 128],
                                             hT[k][:, cb * 512:(cb + 1) * 512],
                                             start=(k == 0), stop=(k == 1))
                        sl_ = (slice(None), slice(cb * 512, (cb + 1) * 512))
                        bcol = slice(l * 2 + m, l * 2 + m + 1)
                        nc.scalar.activation(xlTn[m][sl_], pp[:], AF.Identity,
                                             bias=nblT[:, bcol], scale=-1.0)
                        nc.scalar.activation(xlOb[m][sl_], pp[:], AF.Identity,
                                             bias=oblT[:, bcol])
                    for m in range(2):
                        pp = ppool.tile([128, 512], F32, name=_nm("pp"), tag="pps", bufs=2)
                        for k in range(2):
                            nc.tensor.matmul(pp[:], wr[l][k][:, m * 128:(m + 1) * 128],
                                             hT[k][:, cb * 512:(cb + 1) * 512],
                                             start=(k == 0), stop=(k == 1))
                        nc.scalar.activation(
                            xrTb[m][:, cb * 512:(cb + 1) * 512], pp[:], AF.Identity,
                            bias=brT[:, l * 2 + m:l * 2 + m + 1])

                # ---- w_j = exp(att.xl + mask - WB) via flipped pax ----
                wT = smpool.tile([128, 32], F32, name=_nm("wT"), tag="wT", bufs=2)
                for gp in range(8):
                    paxp = ppool.tile([128, 512], F32, name=_nm("paxp"), tag="ops",
                                      bufs=2)
                    for par in range(2):
                        g = gp * 2 + par
                        for hp in range(2):
                            nc.tensor.matmul(
                                paxp[par * 64:par * 64 + 64, hp * 2:hp * 2 + 2],
                                xlTn[hp][:, g * 64:g * 64 + 64],
                                attN[:, l * 4 + hp * 2:l * 4 + hp * 2 + 2],
                                start=True, stop=True,
                                tile_position=(0, 64 * par))
                    nc.scalar.activation(wT[:, gp * 4:gp * 4 + 4], paxp[:, 0:4],
                                         AF.Exp, bias=mbT[:, gp:gp + 1])

                # ---- xn0 = w * xlOb node-major (PE transpose) + w cols ----
                xn0 = smpool.tile([128, 8 * 320 + 64], F16, name=_nm("xn"), tag="xn0",
                                  bufs=1)
                for gp in range(8):
                    for hp in range(2):
                        tp = ppool.tile([128, 128], F16, name=_nm("tp"), tag="tp",
                                        bufs=2)
                        nc.tensor.transpose(tp[:], xlOb[hp][:, gp * 128:(gp + 1) * 128],
                                            idn[:])
                        evace()(xn0[:, gp * 320 + hp * 128:gp * 320 + hp * 128 + 128],
                                tp[:])
                    evace()(xn0[:, gp * 320 + 256:gp * 320 + 260],
                            wT[:, gp * 4:gp * 4 + 4])
                # scale xlOb rows by w_j: one wide op, per (gp, h) 64-col block
                nc.vector.tensor_tensor(
                    fd(xn0[0:128, 0:1], (320, 8), (64, 4), (1, 64)),
                    fd(xn0[0:128, 0:1], (320, 8), (64, 4), (1, 64)),
                    fd(wT[0:128, 0:1], (4, 8), (1, 4), (0, 64)), op=ALU.mult)

                # ---- attention: E = exp(0.8*att.max - WB) ----
                e_all = wpool.tile([128, etot], F16, name=_nm("eall"), tag="eall")
                for gp in range(8):
                    m = gpm[gp]
                    mm = m * m
                    nch, ipc = _chunking(m)
                    w = ipc * m
                    # sl tiles per (hp): cols par*mm + j*m + i  (j-major)
                    slts = []
                    for hp in range(2):
                        slt = slpool.tile([128, 2 * mm], F16, name=_nm("sl"), tag="sl",
                                          bufs=3, padded_shape=[128, 2 * 64 * 64])
                        for par in range(2):
                            g = gp * 2 + par
                            dst = fd(slt[:, par * mm:par * mm + 1], (m, m), (1, m))
                            xr_sl = xrTb[hp][:, g * 64:g * 64 + 1]
                            xl_sl = xlTn[hp][:, g * 64:g * 64 + 1]
                            maxe().tensor_tensor(dst, fd(xl_sl, (1, m), (0, m)),
                                                 fd(xr_sl, (0, m), (1, m)), op=ALU.max)
                        slts.append(slt)
                    # e matmuls: 4 streams share psum rows 32*s+t; Act evac
                    # fuses exp: E = exp(0.8*pe - WB)
                    for ci in range(nch):
                        pe = ppool.tile([128, 512], F32, name=_nm("pe"),
                                        tag="eps", bufs=2)
                        for hp in range(2):
                            for par in range(2):
                                s = 2 * hp + par
                                nc.tensor.matmul(
                                    pe[32 * s:32 * s + 2, 0:w],
                                    att10[:, (l * 2 + hp) * 32:(l * 2 + hp) * 32 + 2],
                                    slts[hp][:, par * mm + ci * w:
                                             par * mm + (ci + 1) * w],
                                    start=True, stop=True,
                                    tile_position=(0, 32 * s))
                        nc.scalar.activation(
                            e_all[:, eoff[gp] + ci * w:eoff[gp] + (ci + 1) * w],
                            pe[:, 0:w], AF.Exp, bias=ebias[:], scale=0.8)

                # ---- scatter: e_all -> aE_w (per gp; DMA APs cap at 3 dims
                # so the equal-m group merge is not expressible SBUF->SBUF) --
                for gp in range(8):
                    m = gpm[gp]
                    mm = m * m
                    for hp in range(2):
                        for par in range(2):
                            s = 2 * hp + par
                            for t in range(2):
                                src = fd(e_all[32 * s + t:32 * s + t + 1,
                                               eoff[gp]:eoff[gp] + 1],
                                         (m, m), (1, m))
                                cb0 = gp * 512 + (2 * hp + t) * 128 + par * 64
                                db = aE_w[par * 64:par * 64 + m, cb0:cb0 + 1]
                                dstp = fd(db, (1, m))
                                (nc.sync if (hp + par + t + gp) % 2 else
                                 nc.gpsimd).dma_start(dstp, src)

                # ---- attention out (node-major) + Z via w cols ----
                gn16 = wpool.tile([128, 8 * HID], F16, name=_nm("gn16"), tag="gn16")
                rz_w = smpool.tile([128, 32], F32, name=_nm("rzw"), tag="rzw", bufs=2)
                tmin = wpool.tile([128, 8 * HID], F16, name=_nm("tmin"), tag="tmin")
                sum_w = smpool.tile([128, 8], F32, name=_nm("sumw"), tag="sumw", bufs=2)
                vs_w = smpool.tile([128, 8], F32, name=_nm("vsw"), tag="vsw", bufs=2)
                sqs = smpool.tile([128, HID], F16, name=_nm("sqs"), tag="sqs", bufs=2)
                for gp in range(8):
                    po = ppool.tile([128, 512], F32, name=_nm("po"), tag="ops", bufs=2)
                    for h_g in range(4):
                        mov = fd(xn0[0:128, gp * 320 + h_g * 64:gp * 320 + h_g * 64 + 1],
                                 (256 - 63 * h_g, 2), (1, 64))
                        nc.tensor.matmul(
                            po[:, h_g * 128:h_g * 128 + 128],
                            aE_w[:, (gp * 4 + h_g) * 128:(gp * 4 + h_g) * 128 + 128],
                            mov, start=True, stop=True)
                    # clamp Z away from 0 (pad columns i>=m have Z=0); the
                    # clamped rz multiplies an exactly-0 numerator -> 0.
                    zsb = smpool.tile([128, 4], F32, name=_nm("zsb"), tag="zsb",
                                      bufs=2)
                    nc.vector.tensor_scalar(zsb[:], fd(po[0:128, 64:65], (128, 4)),
                                            1e-30, None, op0=ALU.max)
                    nc.vector.reciprocal(rz_w[:, gp * 4:gp * 4 + 4], zsb[:])
                    nc.scalar.activation(
                        gn16[:, gp * HID:gp * HID + HID],
                        fd(po[0:128, 0:1], (128, 4), (1, 64)),
                        AF.Identity)
                    # per-gp tail (pipelines with later gps' attention):
                    # rz mult, ELU (sans -1), LN sums via accum
                    gslc = fd(gn16[0:128, gp * HID:gp * HID + 1], (64, 4), (1, 64))
                    nc.vector.tensor_tensor(
                        gslc, gslc,
                        fd(rz_w[0:128, gp * 4:gp * 4 + 1], (1, 4), (0, 64)),
                        op=ALU.mult)
                    sl8 = slice(gp * HID, (gp + 1) * HID)
                    nc.vector.tensor_scalar(tmin[:, sl8], gn16[:, sl8], 0.0, None,
                                            op0=ALU.min)
                    nc.scalar.activation(tmin[:, sl8], tmin[:, sl8], AF.Exp)
                    nc.vector.scalar_tensor_tensor(
                        gn16[:, sl8], gn16[:, sl8], 0.0, tmin[:, sl8],
                        op0=ALU.max, op1=ALU.add, accum_out=sum_w[:, gp:gp + 1])
                    nc.scalar.activation(sqs[:], gn16[:, sl8], AF.Square,
                                         accum_out=vs_w[:, gp:gp + 1])

                # ---- LayerNorm stats (whole-layer; one Sqrt site/layer
                # keeps Act table swaps to one pair per layer) ----
                mu_w = smpool.tile([128, 8], F32, name=_nm("muw"), tag="muw", bufs=2)
                musq = smpool.tile([128, 8], F32, name=_nm("musq"), tag="musq", bufs=2)
                var_w = smpool.tile([128, 8], F32, name=_nm("varw"), tag="varw", bufs=2)
                rstd_w = smpool.tile([128, 8], F32, name=_nm("rstdw"), tag="rstdw",
                                     bufs=2)
                nmr = smpool.tile([128, 8], F32, name=_nm("nmr"), tag="nmr", bufs=2)
                nc.vector.tensor_scalar(mu_w[:], sum_w[:], 1.0 / HID, None,
                                        op0=ALU.mult)
                nc.vector.tensor_tensor(musq[:], mu_w[:], mu_w[:], op=ALU.mult)
                nc.vector.scalar_tensor_tensor(var_w[:], vs_w[:], 1.0 / HID,
                                               musq[:], op0=ALU.mult,
                                               op1=ALU.subtract)
                nc.scalar.activation(var_w[:], var_w[:], AF.Sqrt, bias=epsb[:])
                nc.vector.reciprocal(rstd_w[:], var_w[:])
                nc.vector.scalar_tensor_tensor(nmr[:], mu_w[:], -1.0, rstd_w[:],
                                               op0=ALU.mult, op1=ALU.mult)
                hn_w = smpool.tile([128, 8 * HID], F16, name=_nm("hn"), tag="hnode",
                                   bufs=2)
                if l + 1 < L:
                    hT = [smpool.tile([128, G * 64], F16, name=_nm("hT"), tag=f"hT{m}",
                                      bufs=1) for m in range(2)]
                for gp in range(8):
                    sl8 = slice(gp * HID, (gp + 1) * HID)
                    nc.scalar.activation(gn16[:, sl8], gn16[:, sl8],
                                         AF.Identity, bias=nmr[:, gp:gp + 1],
                                         scale=rstd_w[:, gp:gp + 1])
                    # contiguous per-gp gamma/beta (2x-eligible), then residual
                    nc.vector.tensor_tensor(gn16[:, sl8], gn16[:, sl8],
                                            gam[l][:, :], op=ALU.mult)
                    nc.vector.tensor_tensor(gn16[:, sl8], gn16[:, sl8],
                                            bet[l][:, :], op=ALU.add)
                    nc.vector.tensor_tensor(hn_w[:, sl8], gn16[:, sl8],
                                            h_node_w[:, sl8], op=ALU.add)
                    if l + 1 < L:
                        for m in range(2):
                            tp = ppool.tile([128, 128], F16, name=_nm("tp"), tag="tp",
                                            bufs=2)
                            nc.tensor.transpose(
                                tp[:],
                                hn_w[:, gp * HID + m * 128:gp * HID + m * 128 + 128],
                                idn[:])
                            evace()(hT[m][:, gp * 128:(gp + 1) * 128],
                                    tp[:])
                h_node_w = hn_w

            # ---------- output DMA ----------
            for par in range(2):
                src = fd(h_node_w[par * 64:par * 64 + 64, 0:1], (HID, 8), (1, HID))
                dst_sl = out_d[par * 64:par * 64 + 1, :]
                dst = bass.AP(dst_sl.tensor, dst_sl.offset,
                              [[HID, 64], [2 * 64 * HID, 8], [1, HID]])
                nc.sync.dma_start(dst, src)

    nc.finalize()
    return nc


# revision 40
# speedup vs baseline: 1.1048x; 1.0028x over previous
"""GATv2Stack Trainium2 kernel (8-core data-parallel over graphs), v3.

bt=128 graphs of N=64 nodes, 16 graphs/core. See reference.py.
  h = x @ W_in + b_in
  2x: xl=h@Wl+bl; xr=h@Wr+br; e=att.lrelu(xr_i+xl_j); a=softmax_j(e+mask)
      g = a@(h@Wl) + (out_bias+bl); g=ELU(g); g=LN(g); h=g+h
  out = where(keep_graph, h, x@W_in+b_in)

v3 design (from v2 trace: Scalar 58%, Vector 57%, DMA queue time ~165us):
  - w-factorization: exp(e'-4) = E_ij * w_j with
      E = exp(0.8*att.max(-xl_j, xr_i) - 2)   [fused into Act psum evac]
      w_j = exp((att.xl)_j + mask_j - 2)      [tiny Act exp of flipped pax]
    attention-out moving operand = w*xlOb (+w cols for Z), so the DVE
    scatter-add (STT) and separate exp pass are deleted entirely.
  - e-scatter DMAs grouped over equal-m gp runs: one DMA per
    (group, head, par, t) instead of per (gp, ...): ~64 -> ~24-32/layer;
    ALL DMAs issued on sync queue only (scalar SEQ freed for Act work).
  - all XBAR DMA transposes (xn0, h_node, hT) -> PE transposes + evacs
  - pairwise-MAX (dominant DVE op) split DVE/GpSimd ~4:3
  - rz (1/Z) folded into gn psum evac as per-partition Act scale
  - ELU's -1 dropped (LN-invariant); sum(x^2) via Act accum_out
Per-core layouts (G=16 graphs, gp pair idx, par=g%2):
  hT[m]     [128,1024] f16  [m*128+c, g*64+node]
  h_node    [128,2048] f16  [par*64+node, gp*256+ch]
  xlTn/xrTb/xlOb[hp] [128,1024] f16 (t,c) x (g,node)
  sl (gp,hp) [128,2*m*m]  f16 cols par*m*m + j*m + i
  e_all      [128, sum(mm)] f16 rows {32s+t}, cols eoff[gp]+par*... E vals
  aE_w      [128,2048] f16  [par*64+j, gp*256 + h*64?? no: gp*512/2..]
            actually [par*64+j, gp*512 + h*128 + par*64 + i] f16 = E
  xn0       [128, 8*320+8] f16 [par*64+node, gp*320 + hp*128 + t*64 + c],
            cols gp*320+256..260 = w_j per head
"""
import sys
sys.path.insert(0, '/opt/trn_rl_repo')
import numpy as np

import concourse.bass as bass
import concourse.mybir as mybir
from concourse import bass_utils, bacc
from concourse.tile import TileContext

dt = mybir.dt
F32, F16 = dt.float32, dt.float16
AF = mybir.ActivationFunctionType
ALU = mybir.AluOpType

B, T, N, D_IN = 2, 64, 64, 512
HID, L, H, C = 256, 2, 4, 64
BT = B * T
G = 16
NCORES = 8
LN_EPS = 1e-5
NEG_BIG = -30000.0
WB = 2.0  # bias split: E=exp(0.8*attmax-2), w=exp(attxl+mask-2)

_n = [0]
def _nm(p="t"):
    _n[0] += 1
    return f"{p}{_n[0]}"


def fd(ap, *dims):
    """Keep partition dim + offset of (sliced) AP, replace free dims."""
    return bass.AP(ap.tensor, ap.offset, [list(ap.ap[0])] + [[s, c] for (s, c) in dims])


def _chunking(m):
    """Uniform i-chunks: smallest even nch with (m/nch)*m <= 512."""
    nch = 2
    while (m // nch) * m > 512 or m % nch != 0:
        nch += 2
    return nch, m // nch


def build_nc(mh=(64,) * G):
    nc = bacc.Bacc("TRN2", target_bir_lowering=False, debug=False,
                   enable_asserts=False, num_devices=1)

    def din(name, shape, dtp=F16):
        return nc.dram_tensor(name, list(shape), dtp, kind="ExternalInput").ap()

    # merged inputs: few big DMAs instead of ~34 small ones
    xT_d    = din("xTm", [128, 4 * G * 64])          # 4 d-chunks side by side
    win_d   = din("winm", [128, 4 * HID])            # 4 d-chunks
    wlr_d   = din("wlrm", [128, 8 * HID])            # wl(l,k) 4x256, wr 4x256
    cf32_d  = din("cf32", [128, 150], F32)  # binT2 nblT4 oblT4 brT4 att128 attN8
    cf16_d  = din("cf16", [128, 2 * HID + 2 * HID + 128])  # gam, bet, idn
    mbT_d   = din("mbT", [128, 8], F32)              # mask - WB, [par*64+j, gp]
    out_d   = nc.dram_tensor("out", [G * 64, HID], F16, kind="ExternalOutput").ap()

    # per-gp m and equal-m groups (consecutive)
    gpm = [mh[2 * gp] for gp in range(8)]
    groups = []
    s0 = 0
    for gp in range(1, 9):
        if gp == 8 or gpm[gp] != gpm[s0]:
            groups.append((s0, gp))
            s0 = gp
    # e_all col layout: per gp block of mm cols = j*m+i; par lives in the
    # psum/e_all ROW (32*(2hp+par)+t), not in a column offset.
    eoff = [0] * 9
    for gp in range(8):
        eoff[gp + 1] = eoff[gp] + gpm[gp] * gpm[gp]
    etot = eoff[8]

    # engine rotation for small psum->sbuf evacs
    evc = [0]
    def evace():
        # psum -> sbuf evacs: GpSimd cannot access PSUM on TRN2
        evc[0] += 1
        return lambda dst, src: nc.scalar.activation(dst, src, AF.Identity)
    def maxe():
        return nc.vector

    with TileContext(nc) as tc:
        with tc.tile_pool(name="const", bufs=1) as cpool, \
             tc.tile_pool(name="wide", bufs=1) as wpool, \
             tc.tile_pool(name="slp", bufs=1) as slpool, \
             tc.tile_pool(name="sm", bufs=2) as smpool, \
             tc.tile_pool(name="psum", bufs=1, space="PSUM") as ppool:

            def ctile(name, dram_ap, shape, dtp=F16, eng=None):
                t0 = cpool.tile(shape, dtp, name=_nm(name))
                (eng or nc.gpsimd).dma_start(t0[:], dram_ap)
                return t0

            winm = ctile("winm", win_d, [128, 4 * HID], eng=nc.sync)
            win = [winm[:, k * HID:(k + 1) * HID] for k in range(4)]
            wlrm = ctile("wlrm", wlr_d, [128, 8 * HID], eng=nc.scalar)
            wl = [[wlrm[:, (l * 2 + k) * HID:(l * 2 + k + 1) * HID]
                   for k in range(2)] for l in range(L)]
            wr = [[wlrm[:, 4 * HID + (l * 2 + k) * HID:
                        4 * HID + (l * 2 + k + 1) * HID]
                   for k in range(2)] for l in range(L)]
            cf32 = ctile("cf32", cf32_d, [128, 150], F32, eng=nc.scalar)
            binT = cf32[:, 0:2]
            nblT = cf32[:, 2:6]
            oblT = cf32[:, 6:10]
            brT = cf32[:, 10:14]
            att10 = cpool.tile([128, 32 * 2 * L], F16, name=_nm("att10"))
            nc.vector.tensor_copy(att10[:], cf32[:, 14:142])
            attN = cpool.tile([128, 4 * L], F16, name=_nm("attN"))
            nc.vector.tensor_copy(attN[:], cf32[:, 142:150])
            cf16 = ctile("cf16", cf16_d, [128, 4 * HID + 128], eng=nc.scalar)
            gam = [cf16[:, l * HID:(l + 1) * HID] for l in range(L)]
            bet = [cf16[:, 2 * HID + l * HID:2 * HID + (l + 1) * HID]
                   for l in range(L)]
            idn = cf16[:, 4 * HID:4 * HID + 128]
            mbT = ctile("mbT", mbT_d, [128, 8], F32)
            epsb = cpool.tile([128, 1], F32, name=_nm("epsb"))
            nc.vector.memset(epsb[:], LN_EPS)
            ebias = cpool.tile([128, 1], F32, name=_nm("ebias"))
            nc.vector.memset(ebias[:], -WB)

            # aE: exp'd logits; cross-par / pad cells must be EXACTLY 0
            # (they sit inside attention-out stationary slabs).
            aE_w = wpool.tile([128, 16 * HID], F16, name=_nm("aew"), tag="aew")
            nc.gpsimd.memset(aE_w[:, 0:2048], 0.0)
            nc.vector.memset(aE_w[:, 2048:4096], 0.0)

            # ---------- input: load xT (d-major), project ----------
            hT = [smpool.tile([128, G * 64], F16, name=_nm("hT"), tag=f"hT{m}", bufs=1)
                  for m in range(2)]
            with tc.tile_pool(name="xtp", bufs=1) as xtpool:
                xTm = xtpool.tile([128, 4 * G * 64], F16, name=_nm("xT"))
                for hh in range(2):
                    nc.sync.dma_start(xTm[:, hh * 2048:(hh + 1) * 2048],
                                      xT_d[:, hh * 2048:(hh + 1) * 2048])
                xT = [xTm[:, k * 1024:(k + 1) * 1024] for k in range(4)]
                for m in range(2):
                    for cb in range(2):
                        ph = ppool.tile([128, 512], F32, name=_nm("ph"), tag="pps", bufs=2)
                        for k in range(4):
                            nc.tensor.matmul(ph[:], win[k][:, m * 128:(m + 1) * 128],
                                             xT[k][:, cb * 512:(cb + 1) * 512],
                                             start=(k == 0), stop=(k == 3))
                        nc.scalar.activation(hT[m][:, cb * 512:(cb + 1) * 512], ph[:],
                                             AF.Identity, bias=binT[:, m:m + 1])

            # h_node via PE transposes
            h_node_w = smpool.tile([128, 8 * HID], F16, name=_nm("hnode"), tag="hnode",
                                   bufs=2)
            for gp in range(8):
                for m in range(2):
                    tp = ppool.tile([128, 128], F16, name=_nm("tp"), tag="tp", bufs=2)
                    nc.tensor.transpose(tp[:], hT[m][:, gp * 128:(gp + 1) * 128],
                                        idn[:])
                    evace()(
                        h_node_w[:, gp * HID + m * 128:gp * HID + m * 128 + 128],
                        tp[:])

            # ---------- layers ----------
            for l in range(L):
                xrTb = [smpool.tile([128, G * 64], F16, name=_nm("xrTb"), tag=f"xrTb{m}",
                                    bufs=1) for m in range(2)]
                xlTn = [smpool.tile([128, G * 64], F16, name=_nm("xlTn"), tag=f"xlTn{m}",
                                    bufs=1) for m in range(2)]
                xlOb = [smpool.tile([128, G * 64], F16, name=_nm("xlOb"), tag=f"xlOb{m}",
                                    bufs=1) for m in range(2)]
                # cb-major: all of chunk cb=0 (gps 0-3) finishes first so the
                # first gps' sl-MAX starts as early as possible
                for cb in range(2):
                    for m in range(2):
                        pp = ppool.tile([128, 512], F32, name=_nm("pp"), tag="pps", bufs=2)
                        for k in range(2):
                            nc.tensor.matmul(pp[:], wl[l][k][:, m * 128:(m + 1) * 128],
                                             hT[k][:, cb * 512:(cb + 1) * 512],
                                             start=(k == 0), stop=(k == 1))
                        sl_ = (slice(None), slice(cb * 512, (cb + 1) * 512))
                        bcol = slice(l * 2 + m, l * 2 + m + 1)
                        nc.scalar.activation(xlTn[m][sl_], pp[:], AF.Identity,
                                             bias=nblT[:, bcol], scale=-1.0)
                        nc.scalar.activation(xlOb[m][sl_], pp[:], AF.Identity,
                                             bias=oblT[:, bcol])
                    for m in range(2):
                        pp = ppool.tile([128, 512], F32, name=_nm("pp"), tag="pps", bufs=2)
                        for k in range(2):
                            nc.tensor.matmul(pp[:], wr[l][k][:, m * 128:(m + 1) * 128],
                                             hT[k][:, cb * 512:(cb + 1) * 512],
                                             start=(k == 0), stop=(k == 1))
                        nc.scalar.activation(
                            xrTb[m][:, cb * 512:(cb + 1) * 512], pp[:], AF.Identity,
                            bias=brT[:, l * 2 + m:l * 2 + m + 1])

                # ---- w_j = exp(att.xl + mask - WB) via flipped pax ----
                wT = smpool.tile([128, 32], F32, name=_nm("wT"), tag="wT", bufs=2)
                for gp in range(8):
                    paxp = ppool.tile([128, 512], F32, name=_nm("paxp"), tag="ops",
                                      bufs=2)
                    for par in range(2):
                        g = gp * 2 + par
                        for hp in range(2):
                            nc.tensor.matmul(
                                paxp[par * 64:par * 64 + 64, hp * 2:hp * 2 + 2],
                                xlTn[hp][:, g * 64:g * 64 + 64],
                                attN[:, l * 4 + hp * 2:l * 4 + hp * 2 + 2],
                                start=True, stop=True,
                                tile_position=(0, 64 * par))
                    nc.scalar.activation(wT[:, gp * 4:gp * 4 + 4], paxp[:, 0:4],
                                         AF.Exp, bias=mbT[:, gp:gp + 1])

                # ---- xn0 = w * xlOb node-major (PE transpose) + w cols ----
                xn0 = smpool.tile([128, 8 * 320 + 64], F16, name=_nm("xn"), tag="xn0",
                                  bufs=1)
                for gp in range(8):
                    for hp in range(2):
                        tp = ppool.tile([128, 128], F16, name=_nm("tp"), tag="tp",
                                        bufs=2)
                        nc.tensor.transpose(tp[:], xlOb[hp][:, gp * 128:(gp + 1) * 128],
                                            idn[:])
                        evace()(xn0[:, gp * 320 + hp * 128:gp * 320 + hp * 128 + 128],
                                tp[:])
                    evace()(xn0[:, gp * 320 + 256:gp * 320 + 260],
                            wT[:, gp * 4:gp * 4 + 4])
                # scale xlOb rows by w_j: one wide op, per (gp, h) 64-col block
                nc.vector.tensor_tensor(
                    fd(xn0[0:128, 0:1], (320, 8), (64, 4), (1, 64)),
                    fd(xn0[0:128, 0:1], (320, 8), (64, 4), (1, 64)),
                    fd(wT[0:128, 0:1], (4, 8), (1, 4), (0, 64)), op=ALU.mult)

                # ---- attention: E = exp(0.8*att.max - WB) ----
                e_all = wpool.tile([128, etot], F16, name=_nm("eall"), tag="eall")
                for gp in range(8):
                    m = gpm[gp]
                    mm = m * m
                    nch, ipc = _chunking(m)
                    w = ipc * m
                    # sl tiles per (hp): cols par*mm + j*m + i  (j-major)
                    slts = []
                    for hp in range(2):
                        slt = slpool.tile([128, 2 * mm], F16, name=_nm("sl"), tag="sl",
                                          bufs=3, padded_shape=[128, 2 * 64 * 64])
                        # both par graphs in one op via a 3-dim (par, j, i) AP:
                        # halves DVE op count / fixed overhead on the critical
                        # pairwise-max
                        base = xlTn[hp][:, gp * 128:gp * 128 + 1]
                        baser = xrTb[hp][:, gp * 128:gp * 128 + 1]
                        dst = fd(slt[:, 0:1], (mm, 2), (m, m), (1, m))
                        nc.vector.tensor_tensor(
                            dst, fd(base, (64, 2), (1, m), (0, m)),
                            fd(baser, (64, 2), (0, m), (1, m)), op=ALU.max)
                        slts.append(slt)
                    # e matmuls: 4 streams share psum rows 32*s+t; Act evac
                    # fuses exp: E = exp(0.8*pe - WB)
                    for ci in range(nch):
                        pe = ppool.tile([128, 512], F32, name=_nm("pe"),
                                        tag="eps", bufs=2)
                        for hp in range(2):
                            for par in range(2):
                                s = 2 * hp + par
                                nc.tensor.matmul(
                                    pe[32 * s:32 * s + 2, 0:w],
                                    att10[:, (l * 2 + hp) * 32:(l * 2 + hp) * 32 + 2],
                                    slts[hp][:, par * mm + ci * w:
                                             par * mm + (ci + 1) * w],
                                    start=True, stop=True,
                                    tile_position=(0, 32 * s))
                        nc.scalar.activation(
                            e_all[:, eoff[gp] + ci * w:eoff[gp] + (ci + 1) * w],
                            pe[:, 0:w], AF.Exp, bias=ebias[:], scale=0.8)

                # ---- scatter: e_all -> aE_w (per gp; DMA APs cap at 3 dims
                # so the equal-m group merge is not expressible SBUF->SBUF) --
                for gp in range(8):
                    m = gpm[gp]
                    mm = m * m
                    for hp in range(2):
                        for par in range(2):
                            s = 2 * hp + par
                            for t in range(2):
                                src = fd(e_all[32 * s + t:32 * s + t + 1,
                                               eoff[gp]:eoff[gp] + 1],
                                         (m, m), (1, m))
                                cb0 = gp * 512 + (2 * hp + t) * 128 + par * 64
                                db = aE_w[par * 64:par * 64 + m, cb0:cb0 + 1]
                                dstp = fd(db, (1, m))
                                (nc.sync if (hp + par + t + gp) % 2 else
                                 nc.gpsimd).dma_start(dstp, src)

                # ---- attention out (node-major) + Z via w cols ----
                gn16 = wpool.tile([128, 8 * HID], F16, name=_nm("gn16"), tag="gn16")
                rz_w = smpool.tile([128, 32], F32, name=_nm("rzw"), tag="rzw", bufs=2)
                tmin = wpool.tile([128, 8 * HID], F16, name=_nm("tmin"), tag="tmin")
                sum_w = smpool.tile([128, 8], F32, name=_nm("sumw"), tag="sumw", bufs=2)
                vs_w = smpool.tile([128, 8], F32, name=_nm("vsw"), tag="vsw", bufs=2)
                sqs = smpool.tile([128, HID], F16, name=_nm("sqs"), tag="sqs", bufs=2)
                for gp in range(8):
                    po = ppool.tile([128, 512], F32, name=_nm("po"), tag="ops", bufs=2)
                    for h_g in range(4):
                        mov = fd(xn0[0:128, gp * 320 + h_g * 64:gp * 320 + h_g * 64 + 1],
                                 (256 - 63 * h_g, 2), (1, 64))
                        nc.tensor.matmul(
                            po[:, h_g * 128:h_g * 128 + 128],
                            aE_w[:, (gp * 4 + h_g) * 128:(gp * 4 + h_g) * 128 + 128],
                            mov, start=True, stop=True)
                    # clamp Z away from 0 (pad columns i>=m have Z=0); the
                    # clamped rz multiplies an exactly-0 numerator -> 0.
                    zsb = smpool.tile([128, 4], F32, name=_nm("zsb"), tag="zsb",
                                      bufs=2)
                    nc.vector.tensor_scalar(zsb[:], fd(po[0:128, 64:65], (128, 4)),
                                            1e-30, None, op0=ALU.max)
                    nc.vector.reciprocal(rz_w[:, gp * 4:gp * 4 + 4], zsb[:])
                    nc.scalar.activation(
                        gn16[:, gp * HID:gp * HID + HID],
                        fd(po[0:128, 0:1], (128, 4), (1, 64)),
                        AF.Identity)
                    # per-gp tail (pipelines with later gps' attention):
                    # rz mult, ELU (sans -1), LN sums via accum
                    gslc = fd(gn16[0:128, gp * HID:gp * HID + 1], (64, 4), (1, 64))
                    nc.vector.tensor_tensor(
                        gslc, gslc,
                        fd(rz_w[0:128, gp * 4:gp * 4 + 1], (1, 4), (0, 64)),
                        op=ALU.mult)
                    sl8 = slice(gp * HID, (gp + 1) * HID)
                    nc.vector.tensor_scalar(tmin[:, sl8], gn16[:, sl8], 0.0, None,
                                            op0=ALU.min)
                    nc.scalar.activation(tmin[:, sl8], tmin[:, sl8], AF.Exp)
                    nc.vector.scalar_tensor_tensor(
                        gn16[:, sl8], gn16[:, sl8], 0.0, tmin[:, sl8],
                        op0=ALU.max, op1=ALU.add, accum_out=sum_w[:, gp:gp + 1])
                    nc.scalar.activation(sqs[:], gn16[:, sl8], AF.Square,
                                         accum_out=vs_w[:, gp:gp + 1])

                # ---- LayerNorm stats (whole-layer; one Sqrt site/layer
                # keeps Act table swaps to one pair per layer) ----
                mu_w = smpool.tile([128, 8], F32, name=_nm("muw"), tag="muw", bufs=2)
                musq = smpool.tile([128, 8], F32, name=_nm("musq"), tag="musq", bufs=2)
                var_w = smpool.tile([128, 8], F32, name=_nm("varw"), tag="varw", bufs=2)
                rstd_w = smpool.tile([128, 8], F32, name=_nm("rstdw"), tag="rstdw",
                                     bufs=2)
                nmr = smpool.tile([128, 8], F32, name=_nm("nmr"), tag="nmr", bufs=2)
                nc.vector.tensor_scalar(mu_w[:], sum_w[:], 1.0 / HID, None,
                                        op0=ALU.mult)
                nc.vector.tensor_tensor(musq[:], mu_w[:], mu_w[:], op=ALU.mult)
                nc.vector.scalar_tensor_tensor(var_w[:], vs_w[:], 1.0 / HID,
                                               musq[:], op0=ALU.mult,
                                               op1=ALU.subtract)
                nc.scalar.activation(var_w[:], var_w[:], AF.Sqrt, bias=epsb[:])
                nc.vector.reciprocal(rstd_w[:], var_w[:])
                nc.vector.scalar_tensor_tensor(nmr[:], mu_w[:], -1.0, rstd_w[:],
                                               op0=ALU.mult, op1=ALU.mult)
                hn_w = smpool.tile([128, 8 * HID], F16, name=_nm("hn"), tag="hnode",
                                   bufs=2)
                if l + 1 < L:
                    hT = [smpool.tile([128, G * 64], F16, name=_nm("hT"), tag=f"hT{m}",
                                      bufs=1) for m in range(2)]
                for gp in range(8):
                    sl8 = slice(gp * HID, (gp + 1) * HID)
                    nc.scalar.activation(gn16[:, sl8], gn16[:, sl8],
                                         AF.Identity, bias=nmr[:, gp:gp + 1],
                                         scale=rstd_w[:, gp:gp + 1])
                    # contiguous per-gp gamma/beta (2x-eligible), then residual
                    nc.vector.tensor_tensor(gn16[:, sl8], gn16[:, sl8],
                                            gam[l][:, :], op=ALU.mult)
                    nc.vector.tensor_tensor(gn16[:, sl8], gn16[:, sl8],
                                            bet[l][:, :], op=ALU.add)
                    nc.vector.tensor_tensor(hn_w[:, sl8], gn16[:, sl8],
                                            h_node_w[:, sl8], op=ALU.add)
                    if l + 1 < L:
                        for m in range(2):
                            tp = ppool.tile([128, 128], F16, name=_nm("tp"), tag="tp",
                                            bufs=2)
                            nc.tensor.transpose(
                                tp[:],
                                hn_w[:, gp * HID + m * 128:gp * HID + m * 128 + 128],
                                idn[:])
                            evace()(hT[m][:, gp * 128:(gp + 1) * 128],
                                    tp[:])
                h_node_w = hn_w

            # ---------- output DMA ----------
            for par in range(2):
                src = fd(h_node_w[par * 64:par * 64 + 64, 0:1], (HID, 8), (1, HID))
                dst_sl = out_d[par * 64:par * 64 + 1, :]
                dst = bass.AP(dst_sl.tensor, dst_sl.offset,
                              [[HID, 64], [2 * 64 * HID, 8], [1, HID]])
                nc.sync.dma_start(dst, src)

    nc.finalize()
    return nc
